# revision 13
# baseline (speedup 1.0000x reference)
"""MoE routing layer on 8 Trainium2 NeuronCores (data-parallel over batch).

Per core (4 samples):
  routing MLP -> cosine sim vs embeddings -> softmax weights wf[4,10]
  w_eff[b] = sum_n wf[b,n] * conv_w[n]  (conv linear in weights ->
  10x fewer conv FLOPs than materializing all expert convs)
  out[b] = conv2d(x[b], w_eff[b]) + b_eff[b]

Conv is 9 shifted fp16 matmuls over the flat 58-wide grid; FOUR 64x64
PE quadrants run concurrently (2 row groups x 2 col groups), covering
2 samples x 2 row-chunks per slot. A 3-phase schedule lets pair-0 conv
start while pair-1's mixed weights are still being computed.
All DMA goes over the two hardware DGE queues (sync + scalar),
balanced; conv weights stream first (they gate the weight-mix chains),
x behind them, outputs (fp16, host-widened) trail per conv group.
"""
import sys

sys.path.insert(0, "/opt/trn_rl_repo")

import numpy as np

import concourse.bass as bass
import concourse.mybir as mybir
from concourse.tile import TileContext

F32 = mybir.dt.float32
F16 = mybir.dt.float16
AF = mybir.ActivationFunctionType
ALU = mybir.AluOpType
AX = mybir.AxisListType

NCORES = 8
BLOC = 4           # samples per core
CIN = 64
COUT = 64
H = W = 58
HW = H * W         # 3364
OH = OW = 56
NB = 10            # experts
EDIM = 64
RSIZE = 512
HID = 128
NTAP = 9
CHUNK = 8          # output rows per chunk
NCH = 7            # 7*8 = 56 output rows
NFREE = CHUNK * W  # 464 <= 512 (one PSUM bank)
TAP_OFF = [dy * W + dx for dy in range(3) for dx in range(3)]
NWARM = 8

# blob16 column layout (fp16, [128, NCOL16])
C_W1 = 0                 # 512 cols: w1 as [128, 4, 128]
C_W2 = C_W1 + 512        # 64 cols
C_RVT = C_W2 + 64        # 16 cols: rvT as [128, 4, 4]
C_EXT = C_RVT + 16       # 32 fp16 cols = 16 f32 cols bitcast
NCOL16 = C_EXT + 32
# f32 view of the EXT block: [128, 16]
# col 0 = b1; col 1 rows 0:64 = b2; cols 2:12 rows 0:64 = embT; cols 12:16 rows 0:4 = eye(4)

SAMPLE_STRIDE = COUT * OH * OW  # 200704


def fix_sync_waits(nc, cap=2):
    """This walrus build allows at most `cap` sem waits per instruction.
    Splice same-engine NoOps carrying the excess waits right before any
    over-subscribed instruction (waits happen earlier => same semantics)."""
    uid = [0]
    for f in nc.m.functions:
        for blk in f.blocks:
            insts = blk.instructions  # live list
            i = 0
            while i < len(insts):
                inst = insts[i]
                si = inst.sync_info
                waits = list(si.on_wait) if si and si.on_wait else []
                icap = 1
                if len(waits) <= icap:
                    i += 1
                    continue
                keep, excess = waits[-icap:], waits[:-icap]
                for k in range(0, len(excess), icap):
                    nop = mybir.InstNoOp(
                        name=f"{inst.name}-wsplit{uid[0]}", ins=[], outs=[]
                    )
                    uid[0] += 1
                    nop.engine = inst.engine
                    nop.sync_info = mybir.SyncInfo(
                        on_wait=excess[k : k + icap], on_update=[]
                    )
                    nc.register_instruction(nop, overwrite=True)
                    insts.insert(i, nop)
                    i += 1
                inst.sync_info = mybir.SyncInfo(
                    on_wait=keep,
                    on_update=list(si.on_update) if si and si.on_update else [],
                )
                i += 1


def build():
    nc = bass.Bass(num_swdge_queues=1)
    x_d = nc.dram_tensor("x", [BLOC, CIN, HW], F16, kind="ExternalInput")
    cwp_d = nc.dram_tensor("cwp", [CIN, NB * NTAP * COUT], F16, kind="ExternalInput")
    blob16_d = nc.dram_tensor("blob16", [128, NCOL16], F16, kind="ExternalInput")
    blob10_d = nc.dram_tensor("blob10", [NB, 384], F32, kind="ExternalInput")
    out_d = nc.dram_tensor("out", [BLOC, COUT, OH, OW], F16, kind="ExternalOutput")

    with TileContext(nc) as tc:
        with (
            tc.tile_pool(name="consts", bufs=1) as consts,
            tc.tile_pool(name="work", bufs=2) as work,
            tc.tile_pool(name="stage", bufs=3) as stpool,
            tc.tile_pool(name="ps", bufs=2, space="PSUM") as pspool,
            tc.tile_pool(name="psconv", bufs=2, space="PSUM") as psconv,
        ):
            # ---------- SBUF constants / inputs ----------
            ones64 = consts.tile([EDIM, 1], F16, tag="ones64")
            nc.vector.memset(ones64[:], 1.0)
            onesR = consts.tile([1, EDIM], F32, tag="onesR")
            nc.vector.memset(onesR[:], 1.0)

            blob16 = consts.tile([128, NCOL16], F16, tag="blob16")
            blob10 = consts.tile([NB, 384], F32, tag="blob10")
            cwp2 = consts.tile([128, NB, NTAP, COUT], F16, tag="cwp2")
            xt = []
            for j in range(2):
                t = consts.tile([128, HW + 4], F16, tag=f"xt{j}")
                nc.vector.memset(t[:, HW : HW + 4], 0.0)
                xt.append(t)

            # DMA dispatch order == per-engine queue order.
            # sync queue: cwp half0 (gates weff chains), x s0, x s2
            nc.sync.dma_start(out=blob16[0:64], in_=blob16_d[0:64])
            nc.sync.dma_start(
                out=cwp2[0:64].rearrange("p n t c -> p (n t c)"), in_=cwp_d[:]
            )
            nc.sync.dma_start(out=xt[0][0:64, 0:HW], in_=x_d[0])
            nc.sync.dma_start(out=xt[0][64:128, 0:HW], in_=x_d[1])
            nc.sync.dma_start(out=xt[1][0:64, 0:HW], in_=x_d[2])
            nc.sync.dma_start(out=xt[1][64:128, 0:HW], in_=x_d[3])
            # scalar queue: blobs (gate routing), cwp half1; x s1/s3 are
            # dispatched later so they don't block the routing ACTs.
            nc.scalar.dma_start(out=blob16[64:128], in_=blob16_d[64:128])
            nc.scalar.dma_start(out=blob10[:], in_=blob10_d[:])
            nc.scalar.dma_start(
                out=cwp2[64:128].rearrange("p n t c -> p (n t c)"), in_=cwp_d[:]
            )

            w1v = blob16[:, C_W1 : C_W1 + 512].rearrange("p (c m) -> p c m", c=4)
            w2v = blob16[:, C_W2 : C_W2 + 64]
            rvTv = blob16[:, C_RVT : C_RVT + 16].rearrange("p (c b) -> p c b", c=4)
            ext = blob16[:, C_EXT : C_EXT + 32].bitcast(F32)
            b1v = ext[:, 0:1]
            b2v = ext[0:EDIM, 1:2]
            embTv = ext[0:EDIM, 2:12]
            id4 = ext[0:4, 12:16]
            cbA = blob10[:, 0:64]
            cbB = blob10[:, 64:128]
            selA = blob10[0:4, 128:256]
            selB = blob10[0:4, 256:384]

            # ---------- emb normalization (independent of rv) ----------
            esq = work.tile([EDIM, NB], F16, tag="esq")
            nc.vector.tensor_mul(esq[:], embTv, embTv)
            nsqE = pspool.tile([1, NB], F32, tag="small")
            nc.tensor.matmul(nsqE[:], ones64[:], esq[:], start=True, stop=True)
            enrm = work.tile([1, NB], F32, tag="enrm")
            nc.scalar.activation(out=enrm[:], in_=nsqE[:], func=AF.Sqrt)
            einv = work.tile([1, NB], F32, tag="einv")
            nc.vector.reciprocal(einv[:], enrm[:])
            ebc = pspool.tile([EDIM, NB], F32, tag="small")
            nc.tensor.matmul(ebc[:], onesR[:], einv[:], start=True, stop=True)
            embnT = work.tile([EDIM, NB], F16, tag="embnT")
            nc.vector.tensor_mul(embnT[:], embTv, ebc[:])

            # ---------- routing MLP (fp16 weights) ----------
            h1 = pspool.tile([HID, BLOC], F32, tag="small")
            for c in range(4):
                nc.tensor.matmul(
                    h1[:], w1v[:, c, :], rvTv[:, c, :], start=(c == 0), stop=(c == 3)
                )
            h1r = work.tile([HID, BLOC], F16, tag="h1r")
            nc.scalar.activation(
                out=h1r[:], in_=h1[:], func=AF.Relu, bias=b1v, scale=1.0
            )
            rps = pspool.tile([EDIM, BLOC], F32, tag="small")
            nc.tensor.matmul(rps[:], w2v, h1r[:], start=True, stop=True)
            rsb = work.tile([EDIM, BLOC], F16, tag="rsb")
            nc.scalar.activation(
                out=rsb[:], in_=rps[:], func=AF.Identity, bias=b2v, scale=1.0
            )

            # ---------- r norm + cosine sim + softmax ----------
            rsq = work.tile([EDIM, BLOC], F16, tag="rsq")
            nc.vector.tensor_mul(rsq[:], rsb[:], rsb[:])
            nsq = pspool.tile([BLOC, 1], F32, tag="small")
            nc.tensor.matmul(nsq[:], rsq[:], ones64[:], start=True, stop=True)
            rnrm = work.tile([BLOC, 1], F32, tag="rnrm")
            nc.scalar.activation(out=rnrm[:], in_=nsq[:], func=AF.Sqrt)
            rinv = work.tile([BLOC, 1], F32, tag="rinv")
            nc.vector.reciprocal(rinv[:], rnrm[:])

            simps = pspool.tile([BLOC, NB], F32, tag="small")
            nc.tensor.matmul(simps[:], rsb[:], embnT[:], start=True, stop=True)
            # |cosine| <= 1 so exp() is safe without max subtraction
            ex = work.tile([BLOC, NB], F32, tag="ex")
            nc.scalar.activation(out=ex[:], in_=simps[:], func=AF.Exp, scale=rinv[:])
            s = work.tile([BLOC, 1], F32, tag="s")
            nc.vector.tensor_reduce(s[:], ex[:], axis=AX.X, op=ALU.add)
            sinv = work.tile([BLOC, 1], F32, tag="sinv")
            nc.vector.reciprocal(sinv[:], s[:])
            wf = work.tile([BLOC, NB], F32, tag="wf")
            nc.vector.tensor_scalar_mul(out=wf[:], in0=ex[:], scalar1=sinv[:])

            # ---------- wfT / per-partition weight broadcast ----------
            wfT_ps = pspool.tile([NB, BLOC], F32, tag="small")
            nc.tensor.transpose(wfT_ps[:], wf[:], id4)
            wfT = work.tile([NB, BLOC], F32, tag="wfT")
            nc.scalar.copy(out=wfT[:], in_=wfT_ps[:])

            wfbc = []
            for j, sel in enumerate((selA, selB)):
                ps = pspool.tile([128, NB], F32, tag="small")
                nc.tensor.matmul(ps[:], sel, wf[:], start=True, stop=True)
                t = consts.tile([128, NB], F32, tag=f"wfbc{j}")
                nc.vector.tensor_scalar_mul(out=t[:], in0=ps[:], scalar1=1.0)
                wfbc.append(t)

            # ---------- PE warmup (fills weff window, warms HAM) ----------
            # lhsT reads wfbc so the scheduler cannot hoist these above the
            # routing matmuls; rhs streams cwp2 (N=464 keeps the PE busy).
            cwpf = cwp2[:].rearrange("p n t c -> p (n t c)")
            warm_ps = pspool.tile([20, 464], F32, tag="warm")
            for _ in range(NWARM):
                nc.tensor.matmul(
                    warm_ps[:], wfbc[0][:].bitcast(F16), cwpf[:, 0:464],
                    start=True, stop=True,
                )

            # ---------- drain biases biasM[128, 6] ----------
            # col: 0=[s0|s2] 1=[s1|s3] 2=[s2|s2] 3=[s3|s3] 4=[s0|s0] 5=[s1|s1]
            bps = pspool.tile([128, 6], F32, tag="small")
            nc.tensor.matmul(
                bps[0:64, 0:4], cbA, wfT[:, 0:4], start=True, stop=True,
                tile_position=(0, 0),
            )
            nc.tensor.matmul(
                bps[0:64, 4:6], cbA, wfT[:, 0:2], start=True, stop=True,
                tile_position=(0, 0),
            )
            nc.tensor.matmul(
                bps[64:128, 0:2], cbB, wfT[:, 2:4], start=True, stop=True,
                tile_position=(0, 64),
            )
            nc.tensor.matmul(
                bps[64:128, 2:4], cbB, wfT[:, 2:4], start=True, stop=True,
                tile_position=(0, 64),
            )
            nc.tensor.matmul(
                bps[64:128, 4:6], cbB, wfT[:, 0:2], start=True, stop=True,
                tile_position=(0, 64),
            )
            biasM = consts.tile([128, 6], F32, tag="biasM")
            nc.scalar.copy(out=biasM[:], in_=bps[:])

            warm_sink = work.tile([1, 1], F32, tag="warm_sink")
            nc.scalar.copy(out=warm_sink[:], in_=warm_ps[0:1, 0:1])

            # ---------- effective conv weights ----------
            # chain A (samples 0/1) on vector, chain B (2/3) on gpsimd so A
            # completes as early as possible and B overlaps pair-A conv.
            weff = []
            scratch = consts.tile([128, NTAP, COUT], F16, tag="scratch")
            for j in range(2):
                t = consts.tile([128, NTAP, COUT], F16, tag=f"weff{j}")
                if j == 0:
                    nc.vector.tensor_scalar_mul(
                        out=scratch[:], in0=cwp2[:, 0], scalar1=wfbc[j][:, 0:1]
                    )
                else:
                    # op1=bypass: out = in0*scalar; in1 only forces this op
                    # to start after chain A has fully completed.
                    nc.vector.scalar_tensor_tensor(
                        out=scratch[:],
                        in0=cwp2[:, 0],
                        scalar=wfbc[j][:, 0:1],
                        in1=weff[0][:],
                        op0=ALU.mult,
                        op1=ALU.bypass,
                    )
                pp = [scratch, t]
                for n in range(1, NB):
                    src_t, dst_t = pp[(n - 1) % 2], pp[n % 2]
                    nc.vector.scalar_tensor_tensor(
                        out=dst_t[:],
                        in0=cwp2[:, n],
                        scalar=wfbc[j][:, n : n + 1],
                        in1=src_t[:],
                        op0=ALU.mult,
                        op1=ALU.add,
                    )
                weff.append(t)

            # ---------- conv: 7 groups x 9 taps x 4 quadrants ----------
            # group = (X tile idx, chunk A, Y tile idx, chunk B, biasA, biasB)
            groups = [
                (0, 0, 0, 1, 4, 5),
                (0, 2, 0, 3, 4, 5),
                (0, 4, 0, 5, 4, 5),
                (0, 6, 1, 0, 0, 1),
                (1, 1, 1, 2, 2, 3),
                (1, 3, 1, 4, 2, 3),
                (1, 5, 1, 6, 2, 3),
            ]
            for gi, (jx, chA, jy, chB, bcA, bcB) in enumerate(groups):
                wX, wY = weff[jx], weff[jy]
                xX, xY = xt[jx], xt[jy]
                psA = psconv.tile([128, NFREE], F32, tag="psA")
                psB = psconv.tile([128, NFREE], F32, tag="psB")
                for t in range(NTAP):
                    offA = chA * CHUNK * W + TAP_OFF[t]
                    offB = chB * CHUNK * W + TAP_OFF[t]
                    st, sp = (t == 0), (t == NTAP - 1)
                    nc.tensor.matmul(
                        psA[0:64], wX[0:64, t], xX[0:64, offA : offA + NFREE],
                        start=st, stop=sp, tile_position=(0, 0),
                    )
                    nc.tensor.matmul(
                        psB[0:64], wX[64:128, t], xX[64:128, offA : offA + NFREE],
                        start=st, stop=sp, tile_position=(64, 0),
                    )
                    nc.tensor.matmul(
                        psA[64:128], wY[0:64, t], xY[0:64, offB : offB + NFREE],
                        start=st, stop=sp, tile_position=(0, 64),
                    )
                    nc.tensor.matmul(
                        psB[64:128], wY[64:128, t], xY[64:128, offB : offB + NFREE],
                        start=st, stop=sp, tile_position=(64, 64),
                    )
                # drain: psA on scalar(ACT), psB on vector(DVE)
                stage = stpool.tile([128, 2, CHUNK, OW], F16, tag="st")
                psAv = psA[:].rearrange("p (r w) -> p r w", w=W)[:, :, 0:OW]
                psBv = psB[:].rearrange("p (r w) -> p r w", w=W)[:, :, 0:OW]
                nc.scalar.activation(
                    out=stage[:, 0], in_=psAv, func=AF.Identity,
                    bias=biasM[:, bcA : bcA + 1], scale=1.0,
                )
                if gi < 5:
                    # vector is still busy with the weight-mix chains; keep
                    # early psB drains off it so PSUM banks recycle promptly
                    nc.scalar.activation(
                        out=stage[:, 1], in_=psBv, func=AF.Identity,
                        bias=biasM[:, bcB : bcB + 1], scale=1.0,
                    )
                else:
                    nc.vector.tensor_scalar_add(
                        out=stage[:, 1], in0=psBv, scalar1=biasM[:, bcB : bcB + 1]
                    )
                # out DMA: one 4D descriptor per partition-half, queues split
                sX0 = 2 * jx  # sample of X half0 (s0 or s2)
                sY0 = 2 * jy
                oA = out_d[sX0 : sX0 + 2, :, chA * CHUNK : chA * CHUNK + CHUNK, :]
                oB = out_d[sY0 : sY0 + 2, :, chB * CHUNK : chB * CHUNK + CHUNK, :]
                nc.sync.dma_start(
                    out=oA.rearrange("s c r w -> c s r w"), in_=stage[0:64]
                )
                nc.sync.dma_start(
                    out=oB.rearrange("s c r w -> c s r w"), in_=stage[64:128]
                )

    fix_sync_waits(nc)
    return nc


_NC = None


def _get_nc():
    global _NC
    if _NC is None:
        _NC = build()
    return _NC


def make_in_maps(inputs):
    x = np.asarray(inputs["x"], dtype=np.float32)
    rvec = np.asarray(inputs["routing_vector"], dtype=np.float32)
    W1 = np.asarray(inputs["W1"], dtype=np.float32)
    b1 = np.asarray(inputs["b1"], dtype=np.float32)
    W2 = np.asarray(inputs["W2"], dtype=np.float32)
    b2 = np.asarray(inputs["b2"], dtype=np.float32)
    emb = np.asarray(inputs["emb"], dtype=np.float32)
    conv_w = np.asarray(inputs["conv_w"], dtype=np.float32)
    conv_b = np.asarray(inputs["conv_b"], dtype=np.float32)

    x16 = np.ascontiguousarray(
        x.reshape(NCORES, BLOC, CIN, HW).astype(np.float16)
    )
    # conv_w[n, co, ci, ky, kx] -> [ci, n, tap, co] fp16
    cwp = np.ascontiguousarray(
        conv_w.transpose(2, 0, 3, 4, 1).reshape(CIN, NB * NTAP * COUT)
    ).astype(np.float16)

    blob = np.zeros((128, NCOL16), np.float16)
    blob[:, C_W1 : C_W1 + 512] = (
        W1.reshape(4, 128, HID).transpose(1, 0, 2).reshape(128, 512)
    ).astype(np.float16)
    blob[:, C_W2 : C_W2 + 64] = W2.astype(np.float16)
    ext = np.zeros((128, 16), np.float32)
    ext[:, 0] = b1
    ext[0:EDIM, 1] = b2
    ext[0:EDIM, 2:12] = emb.T
    ext[0:4, 12:16] = np.eye(4, dtype=np.float32)
    blob[:, C_EXT : C_EXT + 32] = ext.view(np.float16)

    blob10 = np.zeros((NB, 384), np.float32)
    blob10[:, 0:64] = conv_b
    blob10[:, 64:128] = conv_b
    sel = np.zeros((2, 4, 128), np.float32)
    for j in range(2):
        sel[j, 2 * j, 0:64] = 1.0
        sel[j, 2 * j + 1, 64:128] = 1.0
    blob10[0:4, 128:256] = sel[0]
    blob10[0:4, 256:384] = sel[1]

    in_maps = []
    for c in range(NCORES):
        bc = blob.copy()
        rvc = rvec[BLOC * c : BLOC * (c + 1)]  # [4, 512]
        bc[:, C_RVT : C_RVT + 16] = (
            rvc.T.reshape(4, 128, BLOC).transpose(1, 0, 2).reshape(128, 16)
        ).astype(np.float16)
        in_maps.append(
            {
                "x": x16[c],
                "cwp": cwp,
                "blob16": np.ascontiguousarray(bc),
                "blob10": blob10,
            }
        )
    return in_maps


def kernel(**inputs):
    from concourse.bass_utils import run_bass_kernel_spmd

    nc = _get_nc()
    in_maps = make_in_maps(inputs)
    res = run_bass_kernel_spmd(nc, in_maps, core_ids=list(range(NCORES)))
    return np.concatenate(
        [r["out"].astype(np.float32) for r in res.results], axis=0
    )


# revision 14
# speedup vs baseline: 1.0184x; 1.0184x over previous
"""MoE routing layer on 8 Trainium2 NeuronCores (data-parallel over batch).

Per core (4 samples):
  routing MLP -> cosine sim vs embeddings -> softmax weights wf[4,10]
  w_eff[b] = sum_n wf[b,n] * conv_w[n]  (conv linear in weights ->
  10x fewer conv FLOPs than materializing all expert convs)
  out[b] = conv2d(x[b], w_eff[b]) + b_eff[b]

Conv is 9 shifted fp16 matmuls over the flat 58-wide grid; FOUR 64x64
PE quadrants run concurrently (2 row groups x 2 col groups), covering
2 samples x 2 row-chunks per slot. A 3-phase schedule lets pair-0 conv
start while pair-1's mixed weights are still being computed.
All DMA goes over the two hardware DGE queues (sync + scalar),
balanced; conv weights stream first (they gate the weight-mix chains),
x behind them, outputs (fp16, host-widened) trail per conv group.
"""
import sys

sys.path.insert(0, "/opt/trn_rl_repo")

import numpy as np

import concourse.bass as bass
import concourse.mybir as mybir
from concourse.tile import TileContext

F32 = mybir.dt.float32
F16 = mybir.dt.float16
AF = mybir.ActivationFunctionType
ALU = mybir.AluOpType
AX = mybir.AxisListType

NCORES = 8
BLOC = 4           # samples per core
CIN = 64
COUT = 64
H = W = 58
HW = H * W         # 3364
OH = OW = 56
NB = 10            # experts
EDIM = 64
RSIZE = 512
HID = 128
NTAP = 9
CHUNK = 8          # output rows per chunk
NCH = 7            # 7*8 = 56 output rows
NFREE = CHUNK * W  # 464 <= 512 (one PSUM bank)
TAP_OFF = [dy * W + dx for dy in range(3) for dx in range(3)]
NWARM = 8

# blob16 column layout (fp16, [128, NCOL16])
C_W1 = 0                 # 512 cols: w1 as [128, 4, 128]
C_W2 = C_W1 + 512        # 64 cols
C_RVT = C_W2 + 64        # 16 cols: rvT as [128, 4, 4]
C_EXT = C_RVT + 16       # 32 fp16 cols = 16 f32 cols bitcast
NCOL16 = C_EXT + 32
# f32 view of the EXT block: [128, 16]
# col 0 = b1; col 1 rows 0:64 = b2; cols 2:12 rows 0:64 = embT; cols 12:16 rows 0:4 = eye(4)

SAMPLE_STRIDE = COUT * OH * OW  # 200704


def fix_sync_waits(nc, cap=2):
    """This walrus build allows at most `cap` sem waits per instruction.
    Splice same-engine NoOps carrying the excess waits right before any
    over-subscribed instruction (waits happen earlier => same semantics)."""
    uid = [0]
    for f in nc.m.functions:
        for blk in f.blocks:
            insts = blk.instructions  # live list
            i = 0
            while i < len(insts):
                inst = insts[i]
                si = inst.sync_info
                waits = list(si.on_wait) if si and si.on_wait else []
                icap = 1
                if len(waits) <= icap:
                    i += 1
                    continue
                keep, excess = waits[-icap:], waits[:-icap]
                for k in range(0, len(excess), icap):
                    nop = mybir.InstNoOp(
                        name=f"{inst.name}-wsplit{uid[0]}", ins=[], outs=[]
                    )
                    uid[0] += 1
                    nop.engine = inst.engine
                    nop.sync_info = mybir.SyncInfo(
                        on_wait=excess[k : k + icap], on_update=[]
                    )
                    nc.register_instruction(nop, overwrite=True)
                    insts.insert(i, nop)
                    i += 1
                inst.sync_info = mybir.SyncInfo(
                    on_wait=keep,
                    on_update=list(si.on_update) if si and si.on_update else [],
                )
                i += 1


def build():
    nc = bass.Bass(num_swdge_queues=1)
    x_d = nc.dram_tensor("x", [BLOC, CIN, HW], F16, kind="ExternalInput")
    cwp_d = nc.dram_tensor("cwp", [CIN, NB * NTAP * COUT], F16, kind="ExternalInput")
    blob16_d = nc.dram_tensor("blob16", [128, NCOL16], F16, kind="ExternalInput")
    blob10_d = nc.dram_tensor("blob10", [NB, 384], F32, kind="ExternalInput")
    out_d = nc.dram_tensor("out", [BLOC, COUT, OH, OW], F16, kind="ExternalOutput")

    with TileContext(nc) as tc:
        with (
            tc.tile_pool(name="consts", bufs=1) as consts,
            tc.tile_pool(name="work", bufs=2) as work,
            tc.tile_pool(name="stage", bufs=3) as stpool,
            tc.tile_pool(name="ps", bufs=2, space="PSUM") as pspool,
            tc.tile_pool(name="psconv", bufs=2, space="PSUM") as psconv,
        ):
            # ---------- SBUF constants / inputs ----------
            ones64 = consts.tile([EDIM, 1], F16, tag="ones64")
            nc.vector.memset(ones64[:], 1.0)
            onesR = consts.tile([1, EDIM], F32, tag="onesR")
            nc.vector.memset(onesR[:], 1.0)

            blob16 = consts.tile([128, NCOL16], F16, tag="blob16")
            blob10 = consts.tile([NB, 384], F32, tag="blob10")
            cwp2 = consts.tile([128, NB, NTAP, COUT], F16, tag="cwp2")
            xt = []
            for j in range(2):
                t = consts.tile([128, HW + 4], F16, tag=f"xt{j}")
                nc.vector.memset(t[:, HW : HW + 4], 0.0)
                xt.append(t)

            # DMA dispatch order == per-engine queue order.
            # sync queue: cwp half0 (gates weff chains), x s0, x s2
            nc.sync.dma_start(out=blob16[0:64], in_=blob16_d[0:64])
            nc.sync.dma_start(
                out=cwp2[0:64].rearrange("p n t c -> p (n t c)"), in_=cwp_d[:]
            )
            nc.sync.dma_start(out=xt[0][0:64, 0:HW], in_=x_d[0])
            nc.sync.dma_start(out=xt[1][0:64, 0:HW], in_=x_d[2])
            # scalar queue: blobs (gate routing), cwp half1; x s1/s3 are
            # dispatched later so they don't block the routing ACTs.
            nc.scalar.dma_start(out=blob16[64:128], in_=blob16_d[64:128])
            nc.scalar.dma_start(out=blob10[:], in_=blob10_d[:])
            nc.scalar.dma_start(
                out=cwp2[64:128].rearrange("p n t c -> p (n t c)"), in_=cwp_d[:]
            )

            w1v = blob16[:, C_W1 : C_W1 + 512].rearrange("p (c m) -> p c m", c=4)
            w2v = blob16[:, C_W2 : C_W2 + 64]
            rvTv = blob16[:, C_RVT : C_RVT + 16].rearrange("p (c b) -> p c b", c=4)
            ext = blob16[:, C_EXT : C_EXT + 32].bitcast(F32)
            b1v = ext[:, 0:1]
            b2v = ext[0:EDIM, 1:2]
            embTv = ext[0:EDIM, 2:12]
            id4 = ext[0:4, 12:16]
            cbA = blob10[:, 0:64]
            cbB = blob10[:, 64:128]
            selA = blob10[0:4, 128:256]
            selB = blob10[0:4, 256:384]

            # ---------- emb normalization (independent of rv) ----------
            esq = work.tile([EDIM, NB], F16, tag="esq")
            nc.vector.tensor_mul(esq[:], embTv, embTv)
            nsqE = pspool.tile([1, NB], F32, tag="small")
            nc.tensor.matmul(nsqE[:], ones64[:], esq[:], start=True, stop=True)
            enrm = work.tile([1, NB], F32, tag="enrm")
            nc.scalar.activation(out=enrm[:], in_=nsqE[:], func=AF.Sqrt)
            einv = work.tile([1, NB], F32, tag="einv")
            nc.vector.reciprocal(einv[:], enrm[:])
            ebc = pspool.tile([EDIM, NB], F32, tag="small")
            nc.tensor.matmul(ebc[:], onesR[:], einv[:], start=True, stop=True)
            embnT = work.tile([EDIM, NB], F16, tag="embnT")
            nc.vector.tensor_mul(embnT[:], embTv, ebc[:])

            # ---------- routing MLP (fp16 weights) ----------
            h1 = pspool.tile([HID, BLOC], F32, tag="small")
            for c in range(4):
                nc.tensor.matmul(
                    h1[:], w1v[:, c, :], rvTv[:, c, :], start=(c == 0), stop=(c == 3)
                )
            h1r = work.tile([HID, BLOC], F16, tag="h1r")
            nc.scalar.activation(
                out=h1r[:], in_=h1[:], func=AF.Relu, bias=b1v, scale=1.0
            )
            rps = pspool.tile([EDIM, BLOC], F32, tag="small")
            nc.tensor.matmul(rps[:], w2v, h1r[:], start=True, stop=True)
            rsb = work.tile([EDIM, BLOC], F16, tag="rsb")
            nc.scalar.activation(
                out=rsb[:], in_=rps[:], func=AF.Identity, bias=b2v, scale=1.0
            )

            # ---------- r norm + cosine sim + softmax ----------
            rsq = work.tile([EDIM, BLOC], F16, tag="rsq")
            nc.vector.tensor_mul(rsq[:], rsb[:], rsb[:])
            nsq = pspool.tile([BLOC, 1], F32, tag="small")
            nc.tensor.matmul(nsq[:], rsq[:], ones64[:], start=True, stop=True)
            rnrm = work.tile([BLOC, 1], F32, tag="rnrm")
            nc.scalar.activation(out=rnrm[:], in_=nsq[:], func=AF.Sqrt)
            rinv = work.tile([BLOC, 1], F32, tag="rinv")
            nc.vector.reciprocal(rinv[:], rnrm[:])

            simps = pspool.tile([BLOC, NB], F32, tag="small")
            nc.tensor.matmul(simps[:], rsb[:], embnT[:], start=True, stop=True)
            # |cosine| <= 1 so exp() is safe without max subtraction
            ex = work.tile([BLOC, NB], F32, tag="ex")
            nc.scalar.activation(out=ex[:], in_=simps[:], func=AF.Exp, scale=rinv[:])
            s = work.tile([BLOC, 1], F32, tag="s")
            nc.vector.tensor_reduce(s[:], ex[:], axis=AX.X, op=ALU.add)
            sinv = work.tile([BLOC, 1], F32, tag="sinv")
            nc.vector.reciprocal(sinv[:], s[:])
            wf = work.tile([BLOC, NB], F32, tag="wf")
            nc.vector.tensor_scalar_mul(out=wf[:], in0=ex[:], scalar1=sinv[:])

            # x s1/s3 on the scalar HW queue, after the routing ACTs
            nc.scalar.dma_start(out=xt[0][64:128, 0:HW], in_=x_d[1])
            nc.scalar.dma_start(out=xt[1][64:128, 0:HW], in_=x_d[3])

            # ---------- wfT / per-partition weight broadcast ----------
            wfT_ps = pspool.tile([NB, BLOC], F32, tag="small")
            nc.tensor.transpose(wfT_ps[:], wf[:], id4)
            wfT = work.tile([NB, BLOC], F32, tag="wfT")
            nc.scalar.copy(out=wfT[:], in_=wfT_ps[:])

            wfbc = []
            for j, sel in enumerate((selA, selB)):
                ps = pspool.tile([128, NB], F32, tag="small")
                nc.tensor.matmul(ps[:], sel, wf[:], start=True, stop=True)
                t = consts.tile([128, NB], F32, tag=f"wfbc{j}")
                nc.vector.tensor_scalar_mul(out=t[:], in0=ps[:], scalar1=1.0)
                wfbc.append(t)

            # ---------- PE warmup (fills weff window, warms HAM) ----------
            # lhsT reads wfbc so the scheduler cannot hoist these above the
            # routing matmuls; rhs streams cwp2 (N=464 keeps the PE busy).
            cwpf = cwp2[:].rearrange("p n t c -> p (n t c)")
            warm_ps = pspool.tile([20, 464], F32, tag="warm")
            for _ in range(NWARM):
                nc.tensor.matmul(
                    warm_ps[:], wfbc[0][:].bitcast(F16), cwpf[:, 0:464],
                    start=True, stop=True,
                )

            # ---------- drain biases biasM[128, 6] ----------
            # col: 0=[s0|s2] 1=[s1|s3] 2=[s2|s2] 3=[s3|s3] 4=[s0|s0] 5=[s1|s1]
            bps = pspool.tile([128, 6], F32, tag="small")
            nc.tensor.matmul(
                bps[0:64, 0:4], cbA, wfT[:, 0:4], start=True, stop=True,
                tile_position=(0, 0),
            )
            nc.tensor.matmul(
                bps[0:64, 4:6], cbA, wfT[:, 0:2], start=True, stop=True,
                tile_position=(0, 0),
            )
            nc.tensor.matmul(
                bps[64:128, 0:2], cbB, wfT[:, 2:4], start=True, stop=True,
                tile_position=(0, 64),
            )
            nc.tensor.matmul(
                bps[64:128, 2:4], cbB, wfT[:, 2:4], start=True, stop=True,
                tile_position=(0, 64),
            )
            nc.tensor.matmul(
                bps[64:128, 4:6], cbB, wfT[:, 0:2], start=True, stop=True,
                tile_position=(0, 64),
            )
            biasM = consts.tile([128, 6], F32, tag="biasM")
            nc.scalar.copy(out=biasM[:], in_=bps[:])

            warm_sink = work.tile([1, 1], F32, tag="warm_sink")
            nc.scalar.copy(out=warm_sink[:], in_=warm_ps[0:1, 0:1])

            # ---------- effective conv weights ----------
            # chain A (samples 0/1) on vector, chain B (2/3) on gpsimd so A
            # completes as early as possible and B overlaps pair-A conv.
            weff = []
            scratch = consts.tile([128, NTAP, COUT], F16, tag="scratch")
            for j in range(2):
                t = consts.tile([128, NTAP, COUT], F16, tag=f"weff{j}")
                if j == 0:
                    nc.vector.tensor_scalar_mul(
                        out=scratch[:], in0=cwp2[:, 0], scalar1=wfbc[j][:, 0:1]
                    )
                else:
                    # op1=bypass: out = in0*scalar; in1 only forces this op
                    # to start after chain A has fully completed.
                    nc.vector.scalar_tensor_tensor(
                        out=scratch[:],
                        in0=cwp2[:, 0],
                        scalar=wfbc[j][:, 0:1],
                        in1=weff[0][:],
                        op0=ALU.mult,
                        op1=ALU.bypass,
                    )
                pp = [scratch, t]
                for n in range(1, NB):
                    src_t, dst_t = pp[(n - 1) % 2], pp[n % 2]
                    nc.vector.scalar_tensor_tensor(
                        out=dst_t[:],
                        in0=cwp2[:, n],
                        scalar=wfbc[j][:, n : n + 1],
                        in1=src_t[:],
                        op0=ALU.mult,
                        op1=ALU.add,
                    )
                weff.append(t)

            # ---------- conv: 7 groups x 9 taps x 4 quadrants ----------
            # group = (X tile idx, chunk A, Y tile idx, chunk B, biasA, biasB)
            groups = [
                (0, 0, 0, 1, 4, 5),
                (0, 2, 0, 3, 4, 5),
                (0, 4, 0, 5, 4, 5),
                (0, 6, 1, 0, 0, 1),
                (1, 1, 1, 2, 2, 3),
                (1, 3, 1, 4, 2, 3),
                (1, 5, 1, 6, 2, 3),
            ]
            for gi, (jx, chA, jy, chB, bcA, bcB) in enumerate(groups):
                wX, wY = weff[jx], weff[jy]
                xX, xY = xt[jx], xt[jy]
                psA = psconv.tile([128, NFREE], F32, tag="psA")
                psB = psconv.tile([128, NFREE], F32, tag="psB")
                for t in range(NTAP):
                    offA = chA * CHUNK * W + TAP_OFF[t]
                    offB = chB * CHUNK * W + TAP_OFF[t]
                    st, sp = (t == 0), (t == NTAP - 1)
                    nc.tensor.matmul(
                        psA[0:64], wX[0:64, t], xX[0:64, offA : offA + NFREE],
                        start=st, stop=sp, tile_position=(0, 0),
                    )
                    nc.tensor.matmul(
                        psB[0:64], wX[64:128, t], xX[64:128, offA : offA + NFREE],
                        start=st, stop=sp, tile_position=(64, 0),
                    )
                    nc.tensor.matmul(
                        psA[64:128], wY[0:64, t], xY[0:64, offB : offB + NFREE],
                        start=st, stop=sp, tile_position=(0, 64),
                    )
                    nc.tensor.matmul(
                        psB[64:128], wY[64:128, t], xY[64:128, offB : offB + NFREE],
                        start=st, stop=sp, tile_position=(64, 64),
                    )
                # drain: psA on scalar(ACT), psB on vector(DVE)
                stage = stpool.tile([128, 2, CHUNK, OW], F16, tag="st")
                psAv = psA[:].rearrange("p (r w) -> p r w", w=W)[:, :, 0:OW]
                psBv = psB[:].rearrange("p (r w) -> p r w", w=W)[:, :, 0:OW]
                nc.scalar.activation(
                    out=stage[:, 0], in_=psAv, func=AF.Identity,
                    bias=biasM[:, bcA : bcA + 1], scale=1.0,
                )
                if gi < 5:
                    # vector is still busy with the weight-mix chains; keep
                    # early psB drains off it so PSUM banks recycle promptly
                    nc.scalar.activation(
                        out=stage[:, 1], in_=psBv, func=AF.Identity,
                        bias=biasM[:, bcB : bcB + 1], scale=1.0,
                    )
                else:
                    nc.vector.tensor_scalar_add(
                        out=stage[:, 1], in0=psBv, scalar1=biasM[:, bcB : bcB + 1]
                    )
                # out DMA: one 4D descriptor per partition-half, queues split
                sX0 = 2 * jx  # sample of X half0 (s0 or s2)
                sY0 = 2 * jy
                oA = out_d[sX0 : sX0 + 2, :, chA * CHUNK : chA * CHUNK + CHUNK, :]
                oB = out_d[sY0 : sY0 + 2, :, chB * CHUNK : chB * CHUNK + CHUNK, :]
                nc.sync.dma_start(
                    out=oA.rearrange("s c r w -> c s r w"), in_=stage[0:64]
                )
                nc.sync.dma_start(
                    out=oB.rearrange("s c r w -> c s r w"), in_=stage[64:128]
                )

    fix_sync_waits(nc)
    return nc


_NC = None


def _get_nc():
    global _NC
    if _NC is None:
        _NC = build()
    return _NC


def make_in_maps(inputs):
    x = np.asarray(inputs["x"], dtype=np.float32)
    rvec = np.asarray(inputs["routing_vector"], dtype=np.float32)
    W1 = np.asarray(inputs["W1"], dtype=np.float32)
    b1 = np.asarray(inputs["b1"], dtype=np.float32)
    W2 = np.asarray(inputs["W2"], dtype=np.float32)
    b2 = np.asarray(inputs["b2"], dtype=np.float32)
    emb = np.asarray(inputs["emb"], dtype=np.float32)
    conv_w = np.asarray(inputs["conv_w"], dtype=np.float32)
    conv_b = np.asarray(inputs["conv_b"], dtype=np.float32)

    x16 = np.ascontiguousarray(
        x.reshape(NCORES, BLOC, CIN, HW).astype(np.float16)
    )
    # conv_w[n, co, ci, ky, kx] -> [ci, n, tap, co] fp16
    cwp = np.ascontiguousarray(
        conv_w.transpose(2, 0, 3, 4, 1).reshape(CIN, NB * NTAP * COUT)
    ).astype(np.float16)

    blob = np.zeros((128, NCOL16), np.float16)
    blob[:, C_W1 : C_W1 + 512] = (
        W1.reshape(4, 128, HID).transpose(1, 0, 2).reshape(128, 512)
    ).astype(np.float16)
    blob[:, C_W2 : C_W2 + 64] = W2.astype(np.float16)
    ext = np.zeros((128, 16), np.float32)
    ext[:, 0] = b1
    ext[0:EDIM, 1] = b2
    ext[0:EDIM, 2:12] = emb.T
    ext[0:4, 12:16] = np.eye(4, dtype=np.float32)
    blob[:, C_EXT : C_EXT + 32] = ext.view(np.float16)

    blob10 = np.zeros((NB, 384), np.float32)
    blob10[:, 0:64] = conv_b
    blob10[:, 64:128] = conv_b
    sel = np.zeros((2, 4, 128), np.float32)
    for j in range(2):
        sel[j, 2 * j, 0:64] = 1.0
        sel[j, 2 * j + 1, 64:128] = 1.0
    blob10[0:4, 128:256] = sel[0]
    blob10[0:4, 256:384] = sel[1]

    in_maps = []
    for c in range(NCORES):
        bc = blob.copy()
        rvc = rvec[BLOC * c : BLOC * (c + 1)]  # [4, 512]
        bc[:, C_RVT : C_RVT + 16] = (
            rvc.T.reshape(4, 128, BLOC).transpose(1, 0, 2).reshape(128, 16)
        ).astype(np.float16)
        in_maps.append(
            {
                "x": x16[c],
                "cwp": cwp,
                "blob16": np.ascontiguousarray(bc),
                "blob10": blob10,
            }
        )
    return in_maps


def kernel(**inputs):
    from concourse.bass_utils import run_bass_kernel_spmd

    nc = _get_nc()
    in_maps = make_in_maps(inputs)
    res = run_bass_kernel_spmd(nc, in_maps, core_ids=list(range(NCORES)))
    return np.concatenate(
        [r["out"].astype(np.float32) for r in res.results], axis=0
    )
